# revision 12
# baseline (speedup 1.0000x reference)
"""Trainium2 8-core tensor-parallel Llama3-style GQA attention layer.

Problem: B=1, S=2048, D=4096, H=32 Q heads, KVH=8 KV heads, HD=128,
interleaved-pair RoPE (theta=5e5), causal softmax, output projection.

Sharding (Megatron TP-8):
  - core c owns Q heads [4c..4c+3] and KV head c (GQA groups align exactly),
  - x is replicated (passed pre-transposed as xT so the d-contraction sits on
    partitions with no on-device transposes),
  - wq/wk rows are permuted per head (even pair-indices first, then odd) so the
    interleaved RoPE becomes a "rotate-half" that is partition-aligned; the
    permutation cancels inside the q.k dot product,
  - attention runs in transposed layout (scoresT[s2,s1]) so the attention
    output lands as attnT[e, s] which is exactly the layout the output
    projection needs; softmax denominators via a ones-vector matmul,
  - attnT (bf16) is AllGathered across cores in 4 sequence chunks (overlapped
    with compute), and each core computes a 512-wide slice of the output dim
    of wo (column-parallel) => no reduction collective needed.

kernel(**inputs) takes the FULL fp32 inputs and returns the FULL fp32 output.
"""

import sys

sys.path.insert(0, "/opt/trn_rl_repo")

import math

import numpy as np
import ml_dtypes

import concourse.bass as bass  # noqa: F401  (import keeps bass registry happy)
import concourse.mybir as mybir
import concourse.tile as tile
from concourse import bacc
from concourse.bass_utils import run_bass_kernel_spmd
from concourse.masks import make_identity

bf16 = ml_dtypes.bfloat16
F32 = mybir.dt.float32
BF16 = mybir.dt.bfloat16

# Problem shapes (hardcoded per spec)
B, S, D = 1, 2048, 4096
H, KVH, HD = 32, 8, 128
NCORES = 8
HLOC = H // NCORES            # 4 q heads per core
ELOC = HLOC * HD              # 512 attn-out dims per core
NKO = D // 128                # 32 k-tiles of the d contraction
CHUNK = 512                   # s-chunk (matmul free dim / psum bank)
NCHUNK = S // CHUNK           # 4
NB = S // 128                 # 16 s2 blocks
SCALE = 1.0 / math.sqrt(HD)

_NC_CACHE = None


def _build():
    nc = bacc.Bacc(
        "TRN2",
        target_bir_lowering=False,
        debug=False,
        enable_asserts=True,
        num_devices=NCORES,
    )
    xT_e = nc.dram_tensor("xT", [D, S], BF16, kind="ExternalInput")
    wq_e = nc.dram_tensor("wqT", [D, ELOC], BF16, kind="ExternalInput")
    wk_e = nc.dram_tensor("wkT", [D, HD], BF16, kind="ExternalInput")
    wv_e = nc.dram_tensor("wvT", [D, HD], BF16, kind="ExternalInput")
    wo_e = nc.dram_tensor("woT", [D, ELOC], BF16, kind="ExternalInput")
    cos_e = nc.dram_tensor("cosT", [HD, S], F32, kind="ExternalInput")
    sin_e = nc.dram_tensor("sinT", [HD, S], F32, kind="ExternalInput")
    out_e = nc.dram_tensor("out", [ELOC, S], F32, kind="ExternalOutput")

    xT = xT_e.ap().rearrange("(ko p) s -> p ko s", p=128)       # [128, 32, 2048]
    wqT = wq_e.ap().rearrange("(ko p) m -> p ko m", p=128)      # [128, 32, 512]
    wkT = wk_e.ap().rearrange("(ko p) m -> p ko m", p=128)      # [128, 32, 128]
    wvT = wv_e.ap().rearrange("(ko p) m -> p ko m", p=128)
    woT = wo_e.ap().rearrange("(ko p) m -> p ko m", p=128)      # [128, 32, 512]

    rg = [list(range(NCORES))]

    with tile.TileContext(nc) as tc:
        with (
            tc.tile_pool(name="dram", bufs=1, space="DRAM") as dram_pool,
            tc.tile_pool(name="persist", bufs=1) as pp,
        ):
            AGW = 2 * CHUNK  # AllGather chunk width (2 s-chunks per AG)
            ag_in = [
                dram_pool.tile([ELOC, AGW], BF16, name=f"ag_in{j}")
                for j in range(2)
            ]
            ag_out = [
                dram_pool.tile(
                    [NCORES * ELOC, AGW], BF16, name=f"ag_out{j}",
                    addr_space="Shared",
                )
                for j in range(2)
            ]

            # Tiny warmup collective: absorbs first-collective setup cost
            # while the projections run.
            warm_in = dram_pool.tile([128, 16], BF16, name="warm_in")
            warm_out = dram_pool.tile(
                [1024, 16], BF16, name="warm_out", addr_space="Shared"
            )
            nc.gpsimd.collective_compute(
                "AllGather",
                mybir.AluOpType.bypass,
                replica_groups=rg,
                ins=[warm_in[:].opt()],
                outs=[warm_out[:].opt()],
            )

            # ---- small constants ----
            band = pp.tile([128, 896], BF16)
            nc.gpsimd.memset(band[:], 1.0)
            # band[p, u] = 1 iff u >= p + 384
            nc.gpsimd.affine_select(
                out=band[:], in_=band[:],
                compare_op=mybir.AluOpType.is_ge, fill=0.0,
                base=-384, channel_multiplier=-1, pattern=[[1, 896]],
            )
            ones_sb = pp.tile([128, 1], BF16)
            nc.gpsimd.memset(ones_sb[:], 1.0)
            ident = pp.tile([128, 128], BF16)
            make_identity(nc, ident[:])

            cos_sb = pp.tile([128, S], F32)
            sin_sb = pp.tile([128, S], F32)

            # ---- persistent activations ----
            qsb = pp.tile([128, HLOC, S], BF16)     # roped qT per head
            ksb = pp.tile([128, S], BF16)           # roped kT
            vsb = pp.tile([128, NB, HD], BF16)      # v[s2-tile, :, hd]

            with (
                tc.tile_pool(name="wq", bufs=1) as wqp,
                tc.tile_pool(name="wkv", bufs=1) as wkvp,
                tc.tile_pool(name="xch", bufs=1) as xp,
                tc.tile_pool(name="rope", bufs=2) as rp,
                tc.tile_pool(name="pt", bufs=4) as ptp,
                tc.tile_pool(name="misc", bufs=2) as mp,
                tc.tile_pool(name="stage", bufs=3) as stp,
                tc.tile_pool(name="psAB", bufs=1, space="PSUM") as ps,
            ):
                # Weights and x are split into 4 ko-groups of 8 k-tiles each so
                # the first matmuls only wait on the first ~1MB of DMA, not the
                # whole tensor. Loads alternate between the sync (HWDGE) and
                # gpsimd (SWDGE) queue pools to use both.
                def grp_load(pool, dram_t, m, name, eng_of_g):
                    tiles = []
                    for g in range(4):
                        t = pool.tile([128, 8, m], BF16, name=f"{name}{g}")
                        ko = slice(8 * g, 8 * g + 8)
                        eng = eng_of_g(g)
                        half = max(1, 8 // 2)
                        eng.dma_start(t[:, 0:half, :], dram_t[:, 8 * g:8 * g + half, :])
                        eng.dma_start(t[:, half:8, :], dram_t[:, 8 * g + half:8 * g + 8, :])
                        tiles.append(t)
                    return tiles

                def load_xchunk(j, fine=False):
                    js = slice(j * CHUNK, (j + 1) * CHUNK)
                    xc_g = []
                    for g in range(4):
                        t = xp.tile(
                            [128, 8, CHUNK], BF16, tag=f"xc{g}", bufs=2,
                            name=f"xc{j}_{g}",
                        )
                        eng = nc.gpsimd if g % 2 else nc.sync
                        step = 2 if (fine and g == 0) else 4
                        for s0 in range(0, 8, step):
                            eng.dma_start(
                                t[:, s0:s0 + step, :],
                                xT[:, 8 * g + s0:8 * g + s0 + step, js],
                            )
                        xc_g.append(t)
                    return xc_g

                # DMA issue order = queue order: x-chunk0 first (the very first
                # matmuls need it), then wk, cos/sin (rope), wv, wq.
                xc0_g = load_xchunk(0, fine=True)
                wk_g = grp_load(wkvp, wkT, HD, "wk", lambda g: nc.sync)
                for g in range(2):
                    sl = slice(g * 1024, (g + 1) * 1024)
                    nc.sync.dma_start(cos_sb[:, sl], cos_e.ap()[:, sl])
                    nc.sync.dma_start(sin_sb[:, sl], sin_e.ap()[:, sl])
                wv_g = grp_load(wkvp, wvT, HD, "wv", lambda g: nc.sync)
                wq_g = grp_load(wqp, wqT, ELOC, "wq", lambda g: nc.sync)

                def rope(dst01, src_ps, js):
                    """dst01: (ap_lo, ap_hi) bf16 targets [64, 512] each.
                    src_ps: [128, 512] psum holding permuted projection."""
                    tc_t = rp.tile([128, CHUNK], F32, tag="ropec")
                    ts_t = rp.tile([128, CHUNK], F32, tag="ropes")
                    sw_t = rp.tile([128, CHUNK], F32, tag="ropew")
                    nc.vector.tensor_mul(tc_t[:], src_ps[:], cos_sb[:, js])
                    nc.vector.tensor_mul(ts_t[:], src_ps[:], sin_sb[:, js])
                    nc.sync.dma_start(sw_t[0:64, :], ts_t[64:128, :])
                    nc.sync.dma_start(sw_t[64:128, :], ts_t[0:64, :])
                    nc.vector.tensor_sub(dst01[0], tc_t[0:64, :], sw_t[0:64, :])
                    nc.vector.tensor_add(dst01[1], tc_t[64:128, :], sw_t[64:128, :])

                # ---- phase A: projections + rope, all chunks ----
                for j in range(NCHUNK):
                    js = slice(j * CHUNK, (j + 1) * CHUNK)
                    xc_g = xc0_g if j == 0 else load_xchunk(j)

                    # k projection -> roped ksb[:, js]
                    k_ps = ps.tile([128, CHUNK], F32, tag="kq", bufs=2)
                    for ko in range(NKO):
                        nc.tensor.matmul(
                            k_ps[:],
                            wk_g[ko // 8][:, ko % 8, :],
                            xc_g[ko // 8][:, ko % 8, :],
                            start=(ko == 0), stop=(ko == NKO - 1),
                        )
                    rope((ksb[0:64, js], ksb[64:128, js]), k_ps, js)

                    # vT projection -> transpose -> vsb
                    v_ps = ps.tile([128, CHUNK], F32, tag="kq", bufs=2)
                    for ko in range(NKO):
                        nc.tensor.matmul(
                            v_ps[:],
                            wv_g[ko // 8][:, ko % 8, :],
                            xc_g[ko // 8][:, ko % 8, :],
                            start=(ko == 0), stop=(ko == NKO - 1),
                        )
                    vT_sb = mp.tile([128, CHUNK], BF16, tag="vtsb")
                    nc.scalar.activation(
                        vT_sb[:], v_ps[:], mybir.ActivationFunctionType.Copy
                    )
                    for t in range(4):
                        vtr = ps.tile([128, 128], BF16, tag="vtrden", bufs=2)
                        nc.tensor.transpose(
                            vtr[:], vT_sb[:, t * 128:(t + 1) * 128], ident[:]
                        )
                        nc.scalar.activation(
                            vsb[:, 4 * j + t, :], vtr[:],
                            mybir.ActivationFunctionType.Copy,
                        )

                    # q projections (4 heads) -> roped qsb[:, h, js]
                    for h in range(HLOC):
                        q_ps = ps.tile([128, CHUNK], F32, tag="kq", bufs=2)
                        for ko in range(NKO):
                            nc.tensor.matmul(
                                q_ps[:],
                                wq_g[ko // 8][:, ko % 8, h * 128:(h + 1) * 128],
                                xc_g[ko // 8][:, ko % 8, :],
                                start=(ko == 0), stop=(ko == NKO - 1),
                            )
                        rope((qsb[0:64, h, js], qsb[64:128, h, js]), q_ps, js)

                # ---- phase B: attention. Forward chunk order: AG0 triggers
                # ~12us into attention, so the serial collective stream runs
                # concurrently with the remaining attention + wo compute ----
                for j in range(NCHUNK):
                    js = slice(j * CHUNK, (j + 1) * CHUNK)
                    nblk = 4 * (j + 1)
                    for h in range(HLOC):
                        o_ps = ps.tile([128, CHUNK], F32, tag="o", bufs=2)
                        den_ps = ps.tile([1, CHUNK], F32, tag="vtrden", bufs=2)
                        for i in range(nblk):
                            sc_ps = ps.tile([128, CHUNK], F32, tag="sc", bufs=2)
                            nc.tensor.matmul(
                                sc_ps[:],
                                ksb[:, i * 128:(i + 1) * 128],
                                qsb[:, h, js],
                                start=True, stop=True,
                            )
                            pt = ptp.tile([128, CHUNK], BF16, tag="pt")
                            nc.scalar.activation(
                                pt[:], sc_ps[:],
                                mybir.ActivationFunctionType.Exp, scale=SCALE,
                            )
                            t = i - 4 * j
                            if t >= 0:  # diagonal block: zero the s1 < s2 part
                                off = 384 - 128 * t
                                nc.vector.tensor_mul(
                                    pt[:], pt[:], band[:, off:off + CHUNK]
                                )
                            nc.tensor.matmul(
                                o_ps[:], vsb[:, i, :], pt[:],
                                start=(i == 0), stop=(i == nblk - 1),
                            )
                            nc.tensor.matmul(
                                den_ps[:], ones_sb[:], pt[:],
                                start=(i == 0), stop=(i == nblk - 1),
                            )
                        recip = mp.tile([1, CHUNK], F32, tag="recip")
                        nc.vector.reciprocal_approx_fast(recip[:], den_ps[:])
                        rb = mp.tile([128, CHUNK], F32, tag="rb")
                        nc.gpsimd.partition_broadcast(rb[:], recip[:])
                        att = stp.tile([128, CHUNK], BF16, tag="att")
                        nc.vector.tensor_mul(att[:], o_ps[:], rb[:])
                        co = (j % 2) * CHUNK
                        last_attn_inst = nc.sync.dma_start(
                            ag_in[j // 2][h * 128:(h + 1) * 128, co:co + CHUNK],
                            att[:],
                        )
                    if j % 2 == 1:
                        nc.gpsimd.collective_compute(
                            "AllGather",
                            mybir.AluOpType.bypass,
                            replica_groups=rg,
                            ins=[ag_in[j // 2][:].opt()],
                            outs=[ag_out[j // 2][:].opt()],
                        )

            # ---- phase C: output projection (column-parallel) ----
            with (
                tc.tile_pool(name="wo", bufs=1) as wop,
                tc.tile_pool(name="ag", bufs=2) as agp,
                tc.tile_pool(name="ost", bufs=3) as ostp,
                tc.tile_pool(name="psC", bufs=2, space="PSUM") as psc,
            ):
                wo_sb = wop.tile([128, NKO, ELOC], BF16)
                for g in range(8):
                    ko = slice(4 * g, 4 * g + 4)
                    nc.sync.dma_start(wo_sb[:, ko, :], woT[:, ko, :])
                first_wo = True
                for j in range(NCHUNK):
                    js = slice(j * CHUNK, (j + 1) * CHUNK)
                    co = (j % 2) * CHUNK
                    agt = ag_out[j // 2][:].rearrange("(ko p) s -> p ko s", p=128)
                    # split the gathered-attn load into 4 e-groups so the first
                    # wo matmuls start after ~1MB instead of 4MB of DMA
                    aggrp = []
                    for g in range(4):
                        agsb = agp.tile(
                            [128, 8, CHUNK], BF16, tag=f"agsb{g}", bufs=2
                        )
                        nc.gpsimd.dma_start(
                            agsb[:, 0:4, :],
                            agt[:, 8 * g:8 * g + 4, co:co + CHUNK],
                        )
                        nc.gpsimd.dma_start(
                            agsb[:, 4:8, :],
                            agt[:, 8 * g + 4:8 * g + 8, co:co + CHUNK],
                        )
                        aggrp.append(agsb)
                    for t in range(4):
                        wo_ps = psc.tile([128, CHUNK], F32, tag="wo")
                        for ko in range(NKO):
                            mm = nc.tensor.matmul(
                                wo_ps[:],
                                wo_sb[:, ko, t * 128:(t + 1) * 128],
                                aggrp[ko // 8][:, ko % 8, :],
                                start=(ko == 0), stop=(ko == NKO - 1),
                            )
                            if first_wo:
                                # keep the PE stream fully ordered:
                                # all attention before any wo
                                tile.add_dep_helper(
                                    mm.ins, last_attn_inst.ins, sync=False,
                                    reason="attention before wo on PE",
                                )
                                first_wo = False
                        osb = ostp.tile([128, CHUNK], F32, tag="osb")
                        nc.scalar.activation(
                            osb[:], wo_ps[:], mybir.ActivationFunctionType.Copy
                        )
                        nc.sync.dma_start(
                            out_e.ap()[t * 128:(t + 1) * 128, js.start:js.start + 256],
                            osb[:, 0:256],
                        )
                        nc.sync.dma_start(
                            out_e.ap()[t * 128:(t + 1) * 128, js.start + 256:js.stop],
                            osb[:, 256:CHUNK],
                        )

    nc.compile()
    return nc


def _get_nc():
    global _NC_CACHE
    if _NC_CACHE is None:
        _NC_CACHE = _build()
    return _NC_CACHE


_PERM = np.concatenate([np.arange(0, HD, 2), np.arange(1, HD, 2)])


def _prep_inputs(x, freqs_cos, freqs_sin, wq, wk, wv, wo):
    xT = np.ascontiguousarray(x.reshape(S, D).T.astype(bf16))
    cosT = np.ascontiguousarray(
        np.concatenate([freqs_cos.T, freqs_cos.T], axis=0).astype(np.float32)
    )
    sinT = np.ascontiguousarray(
        np.concatenate([freqs_sin.T, freqs_sin.T], axis=0).astype(np.float32)
    )
    in_maps = []
    for c in range(NCORES):
        heads = range(HLOC * c, HLOC * (c + 1))
        wq_c = np.concatenate(
            [wq[h * HD:(h + 1) * HD][_PERM] for h in heads], axis=0
        )  # [512, D] permuted
        wqT_c = np.ascontiguousarray(wq_c.T.astype(bf16))
        wk_c = wk[c * HD:(c + 1) * HD][_PERM]
        wkT_c = np.ascontiguousarray(wk_c.T.astype(bf16))
        wv_c = wv[c * HD:(c + 1) * HD]
        wvT_c = np.ascontiguousarray(wv_c.T.astype(bf16))
        woT_c = np.ascontiguousarray(wo[c * ELOC:(c + 1) * ELOC, :].T.astype(bf16))
        in_maps.append(
            {
                "xT": xT,
                "wqT": wqT_c,
                "wkT": wkT_c,
                "wvT": wvT_c,
                "woT": woT_c,
                "cosT": cosT,
                "sinT": sinT,
            }
        )
    return in_maps


def _run(in_maps, trace=False, trace_cores=None):
    nc = _get_nc()
    return run_bass_kernel_spmd(
        nc,
        in_maps,
        list(range(NCORES)),
        trace=trace,
        trace_cores=trace_cores,
    )


def kernel(x, freqs_cos, freqs_sin, wq, wk, wv, wo):
    x = np.asarray(x, dtype=np.float32)
    in_maps = _prep_inputs(
        x,
        np.asarray(freqs_cos, np.float32),
        np.asarray(freqs_sin, np.float32),
        np.asarray(wq, np.float32),
        np.asarray(wk, np.float32),
        np.asarray(wv, np.float32),
        np.asarray(wo, np.float32),
    )
    res = _run(in_maps)
    out = np.empty((S, D), dtype=np.float32)
    for c in range(NCORES):
        out[:, c * ELOC:(c + 1) * ELOC] = np.asarray(
            res.results[c]["out"], dtype=np.float32
        ).T
    return out.reshape(B, S, D)


# revision 13
# speedup vs baseline: 1.0333x; 1.0333x over previous
"""Trainium2 8-core tensor-parallel Llama3-style GQA attention layer.

Problem: B=1, S=2048, D=4096, H=32 Q heads, KVH=8 KV heads, HD=128,
interleaved-pair RoPE (theta=5e5), causal softmax, output projection.

Sharding (Megatron TP-8):
  - core c owns Q heads [4c..4c+3] and KV head c (GQA groups align exactly),
  - x is replicated (passed pre-transposed as xT so the d-contraction sits on
    partitions with no on-device transposes),
  - wq/wk rows are permuted per head (even pair-indices first, then odd) so the
    interleaved RoPE becomes a "rotate-half" that is partition-aligned; the
    permutation cancels inside the q.k dot product,
  - attention runs in transposed layout (scoresT[s2,s1]) so the attention
    output lands as attnT[e, s] which is exactly the layout the output
    projection needs; softmax denominators via a ones-vector matmul,
  - attnT (bf16) is AllGathered across cores in 2 sequence chunks (overlapped
    with compute), and each core computes a 512-wide slice of the output dim
    of wo (column-parallel) => no reduction collective needed.

kernel(**inputs) takes the FULL fp32 inputs and returns the FULL fp32 output.
"""

import sys

sys.path.insert(0, "/opt/trn_rl_repo")

import math

import numpy as np
import ml_dtypes

import concourse.bass as bass  # noqa: F401
import concourse.mybir as mybir
import concourse.tile as tile
from concourse import bacc
from concourse.bass_utils import run_bass_kernel_spmd
from concourse.masks import make_identity

bf16 = ml_dtypes.bfloat16
F32 = mybir.dt.float32
BF16 = mybir.dt.bfloat16

# Problem shapes (hardcoded per spec)
B, S, D = 1, 2048, 4096
H, KVH, HD = 32, 8, 128
NCORES = 8
HLOC = H // NCORES            # 4 q heads per core
ELOC = HLOC * HD              # 512 attn-out dims per core
NKO = D // 128                # 32 k-tiles of the d contraction
CHUNK = 512                   # s-chunk (matmul free dim / psum bank)
NCHUNK = S // CHUNK           # 4
NB = S // 128                 # 16 s2 blocks
SCALE = 1.0 / math.sqrt(HD)

_NC_CACHE = None


def _build():
    nc = bacc.Bacc(
        "TRN2",
        target_bir_lowering=False,
        debug=False,
        enable_asserts=True,
        num_devices=NCORES,
    )
    xT_e = nc.dram_tensor("xT", [D, S], BF16, kind="ExternalInput")
    wq_e = nc.dram_tensor("wqT", [D, ELOC], BF16, kind="ExternalInput")
    wk_e = nc.dram_tensor("wkT", [D, HD], BF16, kind="ExternalInput")
    wv_e = nc.dram_tensor("wvT", [D, HD], BF16, kind="ExternalInput")
    wo_e = nc.dram_tensor("woT", [D, ELOC], BF16, kind="ExternalInput")
    cos_e = nc.dram_tensor("cosT", [HD, S], F32, kind="ExternalInput")
    sin_e = nc.dram_tensor("sinT", [HD, S], F32, kind="ExternalInput")
    out_e = nc.dram_tensor("out", [ELOC, S], F32, kind="ExternalOutput")

    xT = xT_e.ap().rearrange("(ko p) s -> p ko s", p=128)       # [128, 32, 2048]
    wqT = wq_e.ap().rearrange("(ko p) m -> p ko m", p=128)      # [128, 32, 512]
    wkT = wk_e.ap().rearrange("(ko p) m -> p ko m", p=128)      # [128, 32, 128]
    wvT = wv_e.ap().rearrange("(ko p) m -> p ko m", p=128)
    woT = wo_e.ap().rearrange("(ko p) m -> p ko m", p=128)      # [128, 32, 512]

    rg = [list(range(NCORES))]

    with tile.TileContext(nc) as tc:
        with (
            tc.tile_pool(name="dram", bufs=1, space="DRAM") as dram_pool,
            tc.tile_pool(name="persist", bufs=1) as pp,
        ):
            AGW = 2 * CHUNK  # AllGather chunk width (2 s-chunks per AG)
            ag_in = [
                dram_pool.tile([ELOC, AGW], BF16, name=f"ag_in{j}")
                for j in range(2)
            ]
            ag_out = [
                dram_pool.tile(
                    [NCORES * ELOC, AGW], BF16, name=f"ag_out{j}",
                    addr_space="Shared",
                )
                for j in range(2)
            ]

            # ---- small constants ----
            band = pp.tile([128, 896], BF16)
            nc.gpsimd.memset(band[:], 1.0)
            # band[p, u] = 1 iff u >= p + 384
            nc.gpsimd.affine_select(
                out=band[:], in_=band[:],
                compare_op=mybir.AluOpType.is_ge, fill=0.0,
                base=-384, channel_multiplier=-1, pattern=[[1, 896]],
            )
            ones_sb = pp.tile([128, 1], BF16)
            nc.gpsimd.memset(ones_sb[:], 1.0)
            ident = pp.tile([128, 128], BF16)
            make_identity(nc, ident[:])

            cos_sb = pp.tile([128, S], F32)
            sin_sb = pp.tile([128, S], F32)

            # ---- persistent activations ----
            qsb = pp.tile([128, HLOC, S], BF16)     # roped qT per head
            ksb = pp.tile([128, S], BF16)           # roped kT
            vsb = pp.tile([128, NB, HD], BF16)      # v[s2-tile, :, hd]

            with (
                tc.tile_pool(name="wq", bufs=1) as wqp,
                tc.tile_pool(name="wkv", bufs=1) as wkvp,
                tc.tile_pool(name="xch", bufs=1) as xp,
                tc.tile_pool(name="rope", bufs=2) as rp,
                tc.tile_pool(name="pt", bufs=6) as ptp,
                tc.tile_pool(name="misc", bufs=2) as mp,
                tc.tile_pool(name="stage", bufs=3) as stp,
            ):
                def load_xchunk(j, fine=False):
                    js = slice(j * CHUNK, (j + 1) * CHUNK)
                    xc_g = []
                    for g in range(4):
                        t = xp.tile(
                            [128, 8, CHUNK], BF16, tag=f"xc{g}", bufs=2,
                            name=f"xc{j}_{g}",
                        )
                        eng = nc.gpsimd if g % 2 else nc.sync
                        step = 1 if (fine and g == 0) else (2 if fine else 4)
                        for s0 in range(0, 8, step):
                            eng.dma_start(
                                t[:, s0:s0 + step, :],
                                xT[:, 8 * g + s0:8 * g + s0 + step, js],
                            )
                        xc_g.append(t)
                    return xc_g

                def grp_load(pool, dram_t, m, name, nsplit=2):
                    tiles = []
                    for g in range(4):
                        t = pool.tile([128, 8, m], BF16, name=f"{name}{g}")
                        step = 8 // nsplit
                        for s0 in range(0, 8, step):
                            nc.sync.dma_start(
                                t[:, s0:s0 + step, :],
                                dram_t[:, 8 * g + s0:8 * g + s0 + step, :],
                            )
                        tiles.append(t)
                    return tiles

                # DMA issue order = queue service order. The PE consumes, in
                # order: wk+xc0 (k proj), wq (q projs), wv (v proj); cos/sin
                # feed the DVE ropes which can lag (kq psum bufs absorb it).
                xc0_g = load_xchunk(0, fine=True)
                wk_g = grp_load(wkvp, wkT, HD, "wk", nsplit=4)
                wq_g = grp_load(wqp, wqT, ELOC, "wq", nsplit=2)
                for g in range(4):
                    sl = slice(g * 512, (g + 1) * 512)
                    nc.sync.dma_start(cos_sb[:, sl], cos_e.ap()[:, sl])
                    nc.sync.dma_start(sin_sb[:, sl], sin_e.ap()[:, sl])
                wv_g = grp_load(wkvp, wvT, HD, "wv", nsplit=2)

                def rope(dst01, src_ps, js):
                    """dst01: (ap_lo, ap_hi) bf16 targets [64, 512] each.
                    src_ps: [128, 512] psum holding permuted projection."""
                    tc_t = rp.tile([128, CHUNK], F32, tag="ropec")
                    ts_t = rp.tile([128, CHUNK], F32, tag="ropes")
                    sw_t = rp.tile([128, CHUNK], F32, tag="ropew")
                    nc.vector.tensor_mul(tc_t[:], src_ps[:], cos_sb[:, js])
                    nc.vector.tensor_mul(ts_t[:], src_ps[:], sin_sb[:, js])
                    nc.sync.dma_start(sw_t[0:64, :], ts_t[64:128, :])
                    nc.sync.dma_start(sw_t[64:128, :], ts_t[0:64, :])
                    nc.vector.tensor_sub(dst01[0], tc_t[0:64, :], sw_t[0:64, :])
                    nc.vector.tensor_add(dst01[1], tc_t[64:128, :], sw_t[64:128, :])

                # ---- phase A: projections + rope, all chunks ----
                with tc.tile_pool(name="psA", bufs=1, space="PSUM") as psA:
                    for j in range(NCHUNK):
                        js = slice(j * CHUNK, (j + 1) * CHUNK)
                        xc_g = xc0_g if j == 0 else load_xchunk(j)

                        # k projection -> roped ksb[:, js]
                        k_ps = psA.tile([128, CHUNK], F32, tag="kq", bufs=4)
                        for ko in range(NKO):
                            nc.tensor.matmul(
                                k_ps[:],
                                wk_g[ko // 8][:, ko % 8, :],
                                xc_g[ko // 8][:, ko % 8, :],
                                start=(ko == 0), stop=(ko == NKO - 1),
                            )
                        rope((ksb[0:64, js], ksb[64:128, js]), k_ps, js)

                        # q projections (4 heads) -> roped qsb[:, h, js]
                        for h in range(HLOC):
                            q_ps = psA.tile([128, CHUNK], F32, tag="kq", bufs=4)
                            for ko in range(NKO):
                                nc.tensor.matmul(
                                    q_ps[:],
                                    wq_g[ko // 8][:, ko % 8, h * 128:(h + 1) * 128],
                                    xc_g[ko // 8][:, ko % 8, :],
                                    start=(ko == 0), stop=(ko == NKO - 1),
                                )
                            rope((qsb[0:64, h, js], qsb[64:128, h, js]), q_ps, js)

                        # vT projection -> transpose -> vsb
                        v_ps = psA.tile([128, CHUNK], F32, tag="kq", bufs=4)
                        for ko in range(NKO):
                            nc.tensor.matmul(
                                v_ps[:],
                                wv_g[ko // 8][:, ko % 8, :],
                                xc_g[ko // 8][:, ko % 8, :],
                                start=(ko == 0), stop=(ko == NKO - 1),
                            )
                        vT_sb = mp.tile([128, CHUNK], BF16, tag="vtsb")
                        nc.scalar.activation(
                            vT_sb[:], v_ps[:], mybir.ActivationFunctionType.Copy
                        )
                        for t in range(4):
                            vtr = psA.tile([128, 128], BF16, tag="vtr", bufs=2)
                            nc.tensor.transpose(
                                vtr[:], vT_sb[:, t * 128:(t + 1) * 128], ident[:]
                            )
                            nc.scalar.activation(
                                vsb[:, 4 * j + t, :], vtr[:],
                                mybir.ActivationFunctionType.Copy,
                            )

                # ---- phase B: attention; AG per 2 chunks ----
                with tc.tile_pool(name="psB", bufs=1, space="PSUM") as psB:
                    for j in range(NCHUNK):
                        js = slice(j * CHUNK, (j + 1) * CHUNK)
                        nblk = 4 * (j + 1)
                        for h in range(HLOC):
                            o_ps = psB.tile([128, CHUNK], F32, tag="o", bufs=2)
                            den_ps = psB.tile([1, CHUNK], F32, tag="den", bufs=2)
                            for i in range(nblk):
                                sc_ps = psB.tile(
                                    [128, CHUNK], F32, tag="sc", bufs=4
                                )
                                nc.tensor.matmul(
                                    sc_ps[:],
                                    ksb[:, i * 128:(i + 1) * 128],
                                    qsb[:, h, js],
                                    start=True, stop=True,
                                )
                                pt = ptp.tile([128, CHUNK], BF16, tag="pt")
                                nc.scalar.activation(
                                    pt[:], sc_ps[:],
                                    mybir.ActivationFunctionType.Exp, scale=SCALE,
                                )
                                t = i - 4 * j
                                if t >= 0:  # diagonal block: zero s1 < s2
                                    off = 384 - 128 * t
                                    nc.vector.tensor_mul(
                                        pt[:], pt[:], band[:, off:off + CHUNK]
                                    )
                                nc.tensor.matmul(
                                    o_ps[:], vsb[:, i, :], pt[:],
                                    start=(i == 0), stop=(i == nblk - 1),
                                )
                                nc.tensor.matmul(
                                    den_ps[:], ones_sb[:], pt[:],
                                    start=(i == 0), stop=(i == nblk - 1),
                                )
                            recip = mp.tile([1, CHUNK], F32, tag="recip")
                            nc.vector.reciprocal_approx_fast(recip[:], den_ps[:])
                            rb = mp.tile([128, CHUNK], F32, tag="rb")
                            nc.gpsimd.partition_broadcast(rb[:], recip[:])
                            att = stp.tile([128, CHUNK], BF16, tag="att")
                            nc.vector.tensor_mul(att[:], o_ps[:], rb[:])
                            co = (j % 2) * CHUNK
                            last_attn_inst = nc.sync.dma_start(
                                ag_in[j // 2][h * 128:(h + 1) * 128, co:co + CHUNK],
                                att[:],
                            )
                        if j % 2 == 1:
                            nc.gpsimd.collective_compute(
                                "AllGather",
                                mybir.AluOpType.bypass,
                                replica_groups=rg,
                                ins=[ag_in[j // 2][:].opt()],
                                outs=[ag_out[j // 2][:].opt()],
                            )

            # ---- phase C: output projection (column-parallel) ----
            with (
                tc.tile_pool(name="wo", bufs=1) as wop,
                tc.tile_pool(name="ag", bufs=2) as agp,
                tc.tile_pool(name="ost", bufs=3) as ostp,
                tc.tile_pool(name="psC", bufs=2, space="PSUM") as psc,
            ):
                wo_sb = wop.tile([128, NKO, ELOC], BF16)
                for g in range(8):
                    ko = slice(4 * g, 4 * g + 4)
                    nc.sync.dma_start(wo_sb[:, ko, :], woT[:, ko, :])
                first_wo = True
                for j in range(NCHUNK):
                    js = slice(j * CHUNK, (j + 1) * CHUNK)
                    co = (j % 2) * CHUNK
                    agt = ag_out[j // 2][:].rearrange("(ko p) s -> p ko s", p=128)
                    # split the gathered-attn load into 4 e-groups so the first
                    # wo matmuls start after ~1MB instead of 4MB of DMA
                    aggrp = []
                    for g in range(4):
                        agsb = agp.tile(
                            [128, 8, CHUNK], BF16, tag=f"agsb{g}", bufs=2
                        )
                        nc.sync.dma_start(
                            agsb[:, 0:4, :],
                            agt[:, 8 * g:8 * g + 4, co:co + CHUNK],
                        )
                        nc.sync.dma_start(
                            agsb[:, 4:8, :],
                            agt[:, 8 * g + 4:8 * g + 8, co:co + CHUNK],
                        )
                        aggrp.append(agsb)
                    for t in range(4):
                        wo_ps = psc.tile([128, CHUNK], F32, tag="wo")
                        for ko in range(NKO):
                            mm = nc.tensor.matmul(
                                wo_ps[:],
                                wo_sb[:, ko, t * 128:(t + 1) * 128],
                                aggrp[ko // 8][:, ko % 8, :],
                                start=(ko == 0), stop=(ko == NKO - 1),
                            )
                            if first_wo:
                                # keep the PE stream ordered: all attention
                                # before any wo
                                tile.add_dep_helper(
                                    mm.ins, last_attn_inst.ins, sync=False,
                                    reason="attention before wo on PE",
                                )
                                first_wo = False
                        osb = ostp.tile([128, CHUNK], F32, tag="osb")
                        nc.scalar.activation(
                            osb[:], wo_ps[:], mybir.ActivationFunctionType.Copy
                        )
                        nc.gpsimd.dma_start(
                            out_e.ap()[t * 128:(t + 1) * 128, js], osb[:]
                        )

    nc.compile()
    return nc


def _get_nc():
    global _NC_CACHE
    if _NC_CACHE is None:
        _NC_CACHE = _build()
    return _NC_CACHE


_PERM = np.concatenate([np.arange(0, HD, 2), np.arange(1, HD, 2)])


def _prep_inputs(x, freqs_cos, freqs_sin, wq, wk, wv, wo):
    xT = np.ascontiguousarray(x.reshape(S, D).T.astype(bf16))
    cosT = np.ascontiguousarray(
        np.concatenate([freqs_cos.T, freqs_cos.T], axis=0).astype(np.float32)
    )
    sinT = np.ascontiguousarray(
        np.concatenate([freqs_sin.T, freqs_sin.T], axis=0).astype(np.float32)
    )
    in_maps = []
    for c in range(NCORES):
        heads = range(HLOC * c, HLOC * (c + 1))
        wq_c = np.concatenate(
            [wq[h * HD:(h + 1) * HD][_PERM] for h in heads], axis=0
        )  # [512, D] permuted
        wqT_c = np.ascontiguousarray(wq_c.T.astype(bf16))
        wk_c = wk[c * HD:(c + 1) * HD][_PERM]
        wkT_c = np.ascontiguousarray(wk_c.T.astype(bf16))
        wv_c = wv[c * HD:(c + 1) * HD]
        wvT_c = np.ascontiguousarray(wv_c.T.astype(bf16))
        woT_c = np.ascontiguousarray(wo[c * ELOC:(c + 1) * ELOC, :].T.astype(bf16))
        in_maps.append(
            {
                "xT": xT,
                "wqT": wqT_c,
                "wkT": wkT_c,
                "wvT": wvT_c,
                "woT": woT_c,
                "cosT": cosT,
                "sinT": sinT,
            }
        )
    return in_maps


def _run(in_maps, trace=False, trace_cores=None):
    nc = _get_nc()
    return run_bass_kernel_spmd(
        nc,
        in_maps,
        list(range(NCORES)),
        trace=trace,
        trace_cores=trace_cores,
    )


def kernel(x, freqs_cos, freqs_sin, wq, wk, wv, wo):
    x = np.asarray(x, dtype=np.float32)
    in_maps = _prep_inputs(
        x,
        np.asarray(freqs_cos, np.float32),
        np.asarray(freqs_sin, np.float32),
        np.asarray(wq, np.float32),
        np.asarray(wk, np.float32),
        np.asarray(wv, np.float32),
        np.asarray(wo, np.float32),
    )
    res = _run(in_maps)
    out = np.empty((S, D), dtype=np.float32)
    for c in range(NCORES):
        out[:, c * ELOC:(c + 1) * ELOC] = np.asarray(
            res.results[c]["out"], dtype=np.float32
        ).T
    return out.reshape(B, S, D)
